# revision 21
# baseline (speedup 1.0000x reference)
"""SIR-MLP RK4 scan kernel for 8 Trainium2 cores.

Math (per batch element b):
  5 small MLPs produce params R, gamma, I0, kint, tint.
  beta(t) = sigmoid((t - tint)/1.75) * (beta1 - beta0) + beta0,
  beta0 = R*gamma, beta1 = beta0*kint.
  RK4 with dt=1 over t = 0..364 of SIR:
     dS/dt = -beta*S*I/N,  dI/dt = beta*S*I/N - gamma*I
  output[t, b] = S[t-1, b] - S[t, b]  (0 for t=0), reshaped (-1, 365).

Device strategy (pure data parallel, batch sharded 8 ways, 2048/core):
  - dependent DVE ops have a ~225 ns producer->consumer latency floor
    (write-back drain + sem post), so the scan is restructured for the
    SHORTEST per-stage dependency spine: with state held negated
    (Sh = -S), tables q2 = -beta/(2N) (c-prescaled) and G = gamma/q,
    a stage is  n = q2*I ;  m = Sh - G ;  kI = n*m ;  kS = n*Sh
    whose spine is ysI -> n -> kI -> ysI' (3 edges vs 4 for the naive
    form); m/kS/ysS ride the stall gaps.  k halves are packed in one
    [128,32] tile per stage so the RK4 combine is 4 wide ops, most of
    which also ride gaps.
  - state+history unified: Y_n = [Sh_n|I_n] lives in a [128, 365*32]
    arena; the combine writes slot n+1 directly, so the S history costs
    no extra ops.  Host diffs Sh for dI and un-permutes.
  - batch layout on a core: column b in [0,2048) of xT; scan tiles are
    [128 partitions, 16 cols] with b = j*128 + p  (j = col, p = partition),
    which lets the MLP head write params directly into scan layout via
    one [32,128]-stationary matmul per (net, j).
  - q2/q1 tables are built by ACT in time chunks that overlap the scan;
    G chunks are divided out on DVE right after each ACT chunk (the
    divide also absorbs the cross-engine table wait for the whole
    chunk).  Arena slots are DMA'd out in chunks as they finalize.
  - q1 (stage-3 full-step table) is only needed at odd grid points, so
    it is stored at half size.
"""

import os
import numpy as np

B = 16384
D_IN = 8
H = 32
T = 365
NN = 5
NH = 3
N_CORES = 8
BL = B // N_CORES          # 2048 batch per core
FD = BL // 128             # 16 batch columns per partition
N_POP = 8.6e6
GATE = 7.0 / 4.0
NSTEP = T - 1
NGRID = 2 * NSTEP + 1      # 729 half-step time points

# packed-weights column offsets in wpack [128, WPK]
OFF_W0A = 0              # [8, 128]  W0 nets 0-3, d-major
OFF_W0B = 128            # [8, 32]   W0 net 4
OFF_WHA = 160            # + 160*l   [128, 128] block-diag Wh nets 0-3
OFF_WHB = 288            # + 160*l   [32, 32]   Wh net 4
OFF_WO = 640             # [128, 4]  Wo nets 0-3 block rows
OFF_WOB = 644            # [32, 1]   Wo net 4
OFF_B0A = 645            # [128, 1]
OFF_B0B = 646            # [32, 1]
OFF_BHA = 647            # + 2*l [128, 1]
OFF_BHB = 648            # + 2*l [32, 1]
OFF_BOT = 653            # [128, 80] bo replicated in head layout
OFF_WO3 = 733            # [32, 1] Wo net 3 at base partition 0
OFF_X = 734              # [8, BL]
WPK = 734 + BL

_CACHE = {}


def _table_chunks(ngrid):
    """Grid-index ranges for the ACT table build: a small first chunk so
    the scan starts early, then larger ones that hide under the scan."""
    bounds = [0, 33]
    while bounds[-1] < ngrid:
        bounds.append(min(bounds[-1] + 116, ngrid))
    return list(zip(bounds[:-1], bounds[1:]))


def _build_program(n_steps):
    import concourse.bass as bass
    import concourse.tile as tile
    import concourse.mybir as mybir

    dt_f32 = mybir.dt.float32
    op = mybir.AluOpType
    act = mybir.ActivationFunctionType
    ngrid = 2 * n_steps + 1
    nt = n_steps + 1

    nc = bass.Bass()

    # tgrid carries all grid points then the odd-grid points appended
    tg = nc.declare_dram_parameter("tgridG", [128, ngrid + n_steps], dt_f32,
                                   isOutput=False)
    wpack = nc.declare_dram_parameter("wpack", [128, WPK], dt_f32, isOutput=False)
    sh_out = nc.declare_dram_parameter("dI", [128, nt * 2 * FD], dt_f32,
                                       isOutput=True)

    with tile.TileContext(nc) as tc:  # noqa: SIM117
        with (
            tc.tile_pool(name="wts", bufs=1) as wts,
            tc.tile_pool(name="small", bufs=1) as small,
        ):
            # ---------------- loads ----------------
            wp = wts.tile([128, WPK], dt_f32, tag="wp")
            nc.sync.dma_start(wp[:], wpack[:])
            tgrid_sb = wts.tile([128, ngrid + n_steps], dt_f32, tag="tgrid")
            nc.sync.dma_start(tgrid_sb[:], tg[:])

            x_sb = wp[0:D_IN, OFF_X:OFF_X + BL]
            w0A_sb = wp[0:D_IN, OFF_W0A:OFF_W0A + 128]
            w0B_sb = wp[0:D_IN, OFF_W0B:OFF_W0B + H]
            whA_sb = [wp[:, OFF_WHA + 160 * l:OFF_WHA + 160 * l + 128]
                      for l in range(NH)]
            whB_sb = [wp[0:H, OFF_WHB + 160 * l:OFF_WHB + 160 * l + H]
                      for l in range(NH)]
            wo_sb = wp[:, OFF_WO:OFF_WO + 4]
            woB_sb = wp[0:H, OFF_WOB:OFF_WOB + 1]
            b0A_sb = wp[:, OFF_B0A:OFF_B0A + 1]
            b0B_sb = wp[0:H, OFF_B0B:OFF_B0B + 1]
            bhA_sb = [wp[:, OFF_BHA + 2 * l:OFF_BHA + 2 * l + 1] for l in range(NH)]
            bhB_sb = [wp[0:H, OFF_BHB + 2 * l:OFF_BHB + 2 * l + 1] for l in range(NH)]
            boT_sb = wp[:, OFF_BOT:OFF_BOT + 5 * FD]
            wo3_sb = wp[0:H, OFF_WO3:OFF_WO3 + 1]

            bf16 = mybir.dt.bfloat16
            eo = small.tile([128, 5 * FD], dt_f32, tag="eo")
            sp = small.tile([128, 5 * FD], dt_f32, tag="sp")

            # ---------------- MLP (scoped pools, freed before tables) ------
            with (
                tc.tile_pool(name="mlp", bufs=2) as mlp,
                tc.tile_pool(name="psA", bufs=2, space="PSUM") as psA,
            ):
                h_prev = mlp.tile([128, BL], dt_f32, tag="h")
                hB_prev = mlp.tile([H, BL], dt_f32, tag="hB")
                for half in range(2):
                    hs = slice(half * 1024, (half + 1) * 1024)
                    ps = psA.tile([128, 1024], dt_f32, tag="psmlp")
                    psB = psA.tile([H, 1024], dt_f32, tag="psB")
                    for c in range(2):
                        sl = slice(half * 1024 + c * 512,
                                   half * 1024 + (c + 1) * 512)
                        pl = slice(c * 512, (c + 1) * 512)
                        nc.tensor.matmul(ps[:, pl], w0A_sb, x_sb[:, sl])
                        nc.tensor.matmul(psB[:, pl], w0B_sb, x_sb[:, sl])
                    nc.scalar.activation(h_prev[:, hs], ps[:], act.Tanh,
                                         bias=b0A_sb, scale=1.0)
                    nc.scalar.activation(hB_prev[:, hs], psB[:], act.Tanh,
                                         bias=b0B_sb, scale=1.0)

                for l in range(NH):
                    h_new = mlp.tile([128, BL], dt_f32, tag="h")
                    hB_new = mlp.tile([H, BL], dt_f32, tag="hB")
                    for half in range(2):
                        hs = slice(half * 1024, (half + 1) * 1024)
                        psl = psA.tile([128, 1024], dt_f32, tag="psmlp")
                        pslB = psA.tile([H, 1024], dt_f32, tag="psB")
                        for c in range(2):
                            sl = slice(half * 1024 + c * 512,
                                       half * 1024 + (c + 1) * 512)
                            pl = slice(c * 512, (c + 1) * 512)
                            nc.tensor.matmul(psl[:, pl], whA_sb[l], h_prev[:, sl])
                            nc.tensor.matmul(pslB[:, pl], whB_sb[l],
                                             hB_prev[:, sl])
                        nc.scalar.activation(h_new[:, hs], psl[:], act.Tanh,
                                             bias=bhA_sb[l], scale=1.0)
                        nc.scalar.activation(hB_new[:, hs], pslB[:], act.Tanh,
                                             bias=bhB_sb[l], scale=1.0)
                    h_prev, hB_prev = h_new, hB_new

                # ------- head: params straight into scan layout -------
                psO = psA.tile([128, 1024], dt_f32, tag="psmlp")
                h3 = mlp.tile([H, BL], dt_f32, tag="h3")
                nc.vector.tensor_copy(h3[:], h_prev[96:128, :])
                nc.tensor.ldweights(h_prev[0:1, 0:2].bitcast(bf16))
                nc.tensor.ldweights(hB_prev[0:1, 0:2].bitcast(bf16))
                nc.tensor.ldweights(h3[0:1, 0:2].bitcast(bf16))
                for j in range(FD):
                    bsl = slice(j * 128, (j + 1) * 128)
                    for n in range(3):
                        nc.tensor.matmul(psO[:, n * FD + j:n * FD + j + 1],
                                         h_prev[n * H:(n + 1) * H, bsl],
                                         wo_sb[n * H:(n + 1) * H, n:n + 1])
                    nc.tensor.matmul(psO[:, 3 * FD + j:3 * FD + j + 1],
                                     h3[:, bsl], wo3_sb)
                    nc.tensor.matmul(psO[:, 4 * FD + j:4 * FD + j + 1],
                                     hB_prev[:, bsl], woB_sb)

                # softplus(x + bo) = ln(1 + exp(x + bo))
                nc.vector.tensor_tensor(eo[:], psO[:, 0:5 * FD], boT_sb, op.add)
                nc.scalar.activation(eo[:], eo[:], act.Exp, bias=0.0, scale=1.0)
                nc.scalar.activation(sp[:], eo[:], act.Ln, bias=1.0, scale=1.0)

            spR = sp[:, 0:FD]
            spG = sp[:, FD:2 * FD]
            spI0 = sp[:, 2 * FD:3 * FD]
            spK = sp[:, 3 * FD:4 * FD]
            spT = sp[:, 4 * FD:5 * FD]

            # ------------- derived params (all DVE) -------------
            gamT = small.tile([128, FD], dt_f32, tag="gamT")
            gam2T = small.tile([128, FD], dt_f32, tag="gam2T")
            nAT = small.tile([128, FD], dt_f32, tag="nAT")
            nCT = small.tile([128, FD], dt_f32, tag="nCT")
            nA2T = small.tile([128, FD], dt_f32, tag="nA2T")
            nC2T = small.tile([128, FD], dt_f32, tag="nC2T")
            ntT = small.tile([128, FD], dt_f32, tag="ntT")
            rT = small.tile([128, FD], dt_f32, tag="rT")
            b0T = small.tile([128, FD], dt_f32, tag="b0T")
            V = nc.vector
            V.tensor_scalar_add(gamT[:], spG, 0.1)
            V.tensor_scalar_mul(gam2T[:], gamT[:], 0.5)
            V.tensor_scalar_add(rT[:], spR, 1.5)
            V.tensor_tensor(b0T[:], rT[:], gamT[:], op.mult)       # beta0
            V.tensor_scalar_add(rT[:], spK, -1.0)                  # kint-1
            V.tensor_tensor(nAT[:], b0T[:], rT[:], op.mult)
            V.tensor_scalar_mul(nAT[:], nAT[:], -1.0 / N_POP)
            V.tensor_scalar_mul(nCT[:], b0T[:], -1.0 / N_POP)
            V.tensor_scalar_mul(nA2T[:], nAT[:], 0.5)
            V.tensor_scalar_mul(nC2T[:], nCT[:], 0.5)
            V.tensor_scalar(ntT[:], spT, 20.0, -1.0 / GATE, op.add, op.mult)
            # |G|-table coefficients: |G| = gamma/|q| = exp(-ln(aT*sig + bT))
            # with aT = -nA/gamma, bT = -nC/gamma (argument = beta/(N*gamma) > 0)
            igT = small.tile([128, FD], dt_f32, tag="igT")
            aT = small.tile([128, FD], dt_f32, tag="aT")
            bT = small.tile([128, FD], dt_f32, tag="bT")
            V.reciprocal(igT[:], gamT[:])
            V.tensor_scalar_mul(igT[:], igT[:], -1.0)
            V.tensor_tensor(aT[:], nAT[:], igT[:], op.mult)
            V.tensor_tensor(bT[:], nCT[:], igT[:], op.mult)

            # ------------- tables + arena + scan -------------
            with tc.tile_pool(name="big", bufs=1) as big:
                q2t = big.tile([128, ngrid * FD], dt_f32, tag="q2t")
                q1t = big.tile([128, n_steps * FD], dt_f32, tag="q1t")
                Gt = big.tile([128, ngrid * FD], dt_f32, tag="Gt")
                arena = big.tile([128, nt * 2 * FD], dt_f32, tag="arena")
                sig = small.tile([128, 116], dt_f32, tag="sig")
                q2v = q2t[:].rearrange("p (t j) -> p t j", j=FD)
                q1v = q1t[:].rearrange("p (t j) -> p t j", j=FD)
                Gv = Gt[:].rearrange("p (t j) -> p t j", j=FD)

                # state arena slot 0 = [Sh0 | I0], Sh = -S
                V.tensor_scalar_add(arena[:, 0:FD], spI0, -N_POP)
                V.tensor_copy(arena[:, FD:2 * FD], spI0)

                chunks = _table_chunks(ngrid)
                sigJ = big.tile([128, 116 * FD], dt_f32, tag="sigJ")
                sigJv = sigJ[:].rearrange("p (j t) -> p j t", j=FD)
                pt = small.tile([128, 2], dt_f32, tag="pt")

                # ACT: q2 = -beta/2N (all grids), q1 = -beta/N (odd grids),
                # G = gamma/q via Reciprocal, all straight from the sigmoid.
                # Function-phased so ACT switches tables at most ~4x/chunk.
                def build_chunk(ci):
                    g0, g1 = chunks[ci]
                    w = g1 - g0
                    n0 = g0 // 2        # first n with 2n+1 >= g0
                    n1 = g1 // 2        # last n with 2n+1 < g1 (exclusive)
                    w2 = n1 - n0
                    for j in range(FD):
                        nc.scalar.activation(sigJv[:, j, 0:w],
                                             tgrid_sb[:, g0:g1],
                                             act.Sigmoid, bias=ntT[:, j:j + 1],
                                             scale=1.0)
                    for j in range(FD):
                        nc.scalar.activation(q2v[:, g0:g1, j], sigJv[:, j, 0:w],
                                             act.Identity, bias=nC2T[:, j:j + 1],
                                             scale=nA2T[:, j:j + 1])
                    for j in range(FD):  # in-place: sig -> ln(beta/(N*gamma))
                        nc.scalar.activation(sigJv[:, j, 0:w], sigJv[:, j, 0:w],
                                             act.Ln, bias=bT[:, j:j + 1],
                                             scale=aT[:, j:j + 1])
                    for j in range(FD):  # |G| = exp(-ln(...)) = gamma/|q|
                        nc.scalar.activation(Gv[:, g0:g1, j], sigJv[:, j, 0:w],
                                             act.Exp, bias=0.0, scale=-1.0)
                    if w2 > 0:
                        for j in range(FD):
                            nc.scalar.activation(sigJv[:, j, 0:w2],
                                                 tgrid_sb[:, ngrid + n0:ngrid + n1],
                                                 act.Sigmoid,
                                                 bias=ntT[:, j:j + 1], scale=1.0)
                        for j in range(FD):
                            nc.scalar.activation(q1v[:, n0:n1, j],
                                                 sigJv[:, j, 0:w2],
                                                 act.Identity,
                                                 bias=nCT[:, j:j + 1],
                                                 scale=nAT[:, j:j + 1])
                    # 1-col DVE pre-touch of the chunk's last-written block:
                    # absorbs the ACT->DVE wait once for the whole chunk
                    V.tensor_copy(pt[:, 0:1], q1v[:, max(n1 - 1, 0), 15:16])
                    V.tensor_copy(pt[:, 1:2], Gv[:, g1 - 1, 15:16])

                build_chunk(0)

                # ------------- RK4 scan (DVE) -------------
                nn = small.tile([128, FD], dt_f32, tag="nn")
                m = small.tile([128, FD], dt_f32, tag="m")
                K1 = small.tile([128, 2 * FD], dt_f32, tag="K1")
                K2 = small.tile([128, 2 * FD], dt_f32, tag="K2")
                K3 = small.tile([128, 2 * FD], dt_f32, tag="K3")
                K4 = small.tile([128, 2 * FD], dt_f32, tag="K4")
                ys = small.tile([128, 2 * FD], dt_f32, tag="ys")
                c1 = small.tile([128, 2 * FD], dt_f32, tag="c1")
                cc = small.tile([128, 2 * FD], dt_f32, tag="cc")
                qq = small.tile([128, 2 * FD], dt_f32, tag="qq")

                def q2b(g):
                    return q2t[:, g * FD:(g + 1) * FD]

                def q1b(n):
                    return q1t[:, n * FD:(n + 1) * FD]

                def gb(g):
                    return Gt[:, g * FD:(g + 1) * FD]

                # DMA the arena out in slot chunks as they finalize
                dma_bounds = [0, 74, 147, 220, 293, nt]
                dma_next = 1
                next_chunk = 1

                for n in range(n_steps):
                    # make the next table chunk available before it's needed
                    if next_chunk < len(chunks) and 2 * n + 2 >= chunks[next_chunk][0] - 20:
                        build_chunk(next_chunk)
                        next_chunk += 1

                    g0, gh, g1 = 2 * n, 2 * n + 1, 2 * n + 2
                    Yn = arena[:, 32 * n:32 * n + 32]
                    YnS = arena[:, 32 * n:32 * n + 16]
                    YnI = arena[:, 32 * n + 16:32 * n + 32]
                    Yp = arena[:, 32 * (n + 1):32 * (n + 1) + 32]
                    ysS = ys[:, 0:FD]
                    ysI = ys[:, FD:2 * FD]

                    def stage(qblk, gblk, Kk, srcS, srcI, upd):
                        # K = [n*S | n*m]; n = q*I ; m = S - G = Sh + |G|
                        V.tensor_tensor(nn[:], qblk, srcI, op.mult)
                        V.tensor_tensor(m[:], srcS, gblk, op.add)
                        V.tensor_tensor(Kk[:, FD:2 * FD], nn[:], m[:], op.mult)
                        V.tensor_tensor(Kk[:, 0:FD], nn[:], srcS, op.mult)
                        if upd:
                            V.tensor_tensor(ysI, Kk[:, FD:2 * FD], YnI, op.add)
                            V.tensor_tensor(ysS, Kk[:, 0:FD], YnS, op.add)

                    stage(q2b(g0), gb(g0), K1, YnS, YnI, True)
                    stage(q2b(gh), gb(gh), K2, ysS, ysI, True)
                    stage(q1b(n), gb(gh), K3, ysS, ysI, True)
                    # riders: c1/cc/qq slot into stage-4's stall gaps
                    V.scalar_tensor_tensor(c1[:], K2[:], 2.0, K1[:],
                                           op.mult, op.add)
                    V.tensor_tensor(nn[:], q2b(g1), ysI, op.mult)
                    V.tensor_tensor(m[:], ysS, gb(g1), op.add)
                    V.tensor_tensor(cc[:], c1[:], K3[:], op.add)
                    V.tensor_tensor(K4[:, FD:2 * FD], nn[:], m[:], op.mult)
                    V.tensor_tensor(K4[:, 0:FD], nn[:], ysS, op.mult)
                    V.scalar_tensor_tensor(qq[:], cc[:], 1.0 / 3.0, Yn,
                                           op.mult, op.add)
                    # Y_{n+1} = Yn + (K1 + 2K2 + K3 + K4)/3 -> arena slot n+1
                    V.scalar_tensor_tensor(Yp, K4[:], 1.0 / 3.0, qq[:],
                                           op.mult, op.add)

                    if dma_next < len(dma_bounds) and n + 2 == dma_bounds[dma_next]:
                        a = dma_bounds[dma_next - 1] * 2 * FD
                        b = dma_bounds[dma_next] * 2 * FD
                        nc.sync.dma_start(sh_out[:, a:b], arena[:, a:b])
                        dma_next += 1

    _dedup_cross_waits(nc, mybir)
    _split_multi_waits(nc, mybir)
    return nc


_COMPUTE_INSTS = {
    "InstTensorTensor", "InstTensorScalarPtr", "InstTensorScalar",
    "InstTensorReduce", "InstActivation", "InstMatmult", "InstLdweights",
    "InstMemset", "InstTensorCopy", "InstCopy", "InstDrain",
    "InstEventSemaphore", "InstNoOp", "InstRegisterMove", "InstSelect",
    "InstIota", "InstRegisterAlu",
}


def _sem_usage(nc):
    use = {}
    for f in nc.m.functions:
        for bb in f.blocks:
            for ins in bb.instructions:
                si = ins.sync_info
                if si is None:
                    continue
                for w in si.on_wait:
                    use.setdefault(w.id, ([], []))[0].append((ins, w))
                for u in si.on_update:
                    use.setdefault(u.id, ([], []))[1].append((ins, u))
    return use


def _qualifying_sems(nc):
    """Monotonic sems updated by compute instructions of a single engine."""
    out = {}
    for sem_id, (waits, upds) in _sem_usage(nc).items():
        if not upds:
            continue
        engs = {str(i.engine) for i, _ in upds}
        if len(engs) != 1:
            continue
        if not all(type(i).__name__ in _COMPUTE_INSTS for i, _ in upds):
            continue
        if not all(u.update_mode == "sem-inc" and u.update_value == 1
                   and u.update_reg is None for _, u in upds):
            continue
        if not all(w.wait_mode == "sem-ge-imm" and w.wait_reg is None
                   for _, w in waits):
            continue
        out[sem_id] = engs.pop()
    return out


def _dedup_cross_waits(nc, mybir):
    """Drop waits already implied by an earlier wait on the same engine:
    once engine E observed monotonic sem >= v, every later E instruction
    inherits that bound through program order."""
    qual = _qualifying_sems(nc)
    for f in nc.m.functions:
        seen = {}
        for bb in f.blocks:
            for ins in bb.instructions:
                si = ins.sync_info
                if si is None or not si.on_wait:
                    continue
                eng = str(ins.engine)
                kept = []
                for w in si.on_wait:
                    if w.id in qual:
                        prev = seen.get((eng, w.id), -1)
                        if w.wait_value <= prev:
                            continue
                        seen[(eng, w.id)] = w.wait_value
                    kept.append(w)
                if len(kept) != len(si.on_wait):
                    ins.sync_info = mybir.SyncInfo(
                        on_wait=kept, on_update=list(si.on_update))


def _split_multi_waits(nc, mybir):
    """walrus accepts at most one sync wait per instruction: hoist extra
    waits onto same-engine NoOps placed just before the instruction."""
    for f in nc.m.functions:
        for bb in f.blocks:
            insts = list(bb.instructions)
            out = []
            changed = False
            for ins in insts:
                si = ins.sync_info
                if si is not None and len(si.on_wait) > 1:
                    waits = list(si.on_wait)
                    for wt in waits[:-1]:
                        nop = mybir.InstNoOp(
                            name=nc.get_next_instruction_name(),
                            engine=ins.engine,
                            ins=[], outs=[],
                            sync_info=mybir.SyncInfo(on_wait=[wt], on_update=[]),
                        )
                        out.append(nop)
                    changed = True
                    ins.sync_info = mybir.SyncInfo(on_wait=[waits[-1]],
                                                   on_update=list(si.on_update))
                out.append(ins)
            if changed:
                bb.instructions = out


def _host_prep(inputs, n_steps):
    data = np.ascontiguousarray(np.asarray(inputs["data"], np.float32))
    W0 = np.asarray(inputs["W0"], np.float32)
    b0 = np.asarray(inputs["b0"], np.float32)
    Wh = np.asarray(inputs["Wh"], np.float32)
    bh = np.asarray(inputs["bh"], np.float32)
    Wo = np.asarray(inputs["Wo"], np.float32)
    bo = np.asarray(inputs["bo"], np.float32)

    ngrid = 2 * n_steps + 1
    grid = (np.arange(ngrid, dtype=np.float64) * 0.5).astype(np.float32)
    gall = (grid / np.float32(GATE)).astype(np.float32)
    godd = gall[1::2]                              # odd grid points appended
    tgridG = np.ascontiguousarray(np.broadcast_to(
        np.concatenate([gall, godd]), (128, ngrid + n_steps)))

    wpk = np.zeros((128, WPK), np.float32)
    wpk[0:D_IN, OFF_W0A:OFF_W0A + 128] = \
        W0[0:4].transpose(2, 0, 1).reshape(D_IN, 4 * H)
    wpk[0:D_IN, OFF_W0B:OFF_W0B + H] = W0[4].T
    for l in range(NH):
        for n in range(4):
            wpk[n * H:(n + 1) * H, OFF_WHA + 160 * l + n * H:
                OFF_WHA + 160 * l + (n + 1) * H] = Wh[n, l].T
        wpk[0:H, OFF_WHB + 160 * l:OFF_WHB + 160 * l + H] = Wh[4, l].T
        wpk[:, OFF_BHA + 2 * l] = bh[0:4, l].reshape(128)
        wpk[0:H, OFF_BHB + 2 * l] = bh[4, l]
    for n in range(4):
        wpk[n * H:(n + 1) * H, OFF_WO + n] = Wo[n, 0]
    wpk[0:H, OFF_WO3] = Wo[3, 0]
    wpk[0:H, OFF_WOB] = Wo[4, 0]
    wpk[:, OFF_B0A] = b0[0:4].reshape(128)
    wpk[0:H, OFF_B0B] = b0[4]
    for n in range(NN):
        wpk[:, OFF_BOT + n * FD:OFF_BOT + (n + 1) * FD] = bo[n, 0]

    in_maps = []
    for c in range(N_CORES):
        m = {"tgridG": tgridG}
        w = wpk.copy()
        w[0:D_IN, OFF_X:OFF_X + BL] = data[c * BL:(c + 1) * BL].T
        m["wpack"] = w
        in_maps.append(m)
    return in_maps


def _assemble(results, n_steps):
    nt = n_steps + 1
    full = np.empty((nt, N_CORES * BL), np.float32)
    for c in range(N_CORES):
        arr = results[c]["dI"].reshape(128, nt, 2 * FD)
        sh = arr[:, :, 0:FD]                       # Sh[p, t, j], Sh = -S
        dsh = np.concatenate(
            [np.zeros((128, 1, FD), np.float32), sh[:, 1:] - sh[:, :-1]],
            axis=1)                                # dI[t] = Sh[t] - Sh[t-1]
        # batch col b = j*128 + p
        full[:, c * BL:(c + 1) * BL] = dsh.transpose(1, 2, 0).reshape(nt, BL)
    return full


def kernel(**inputs):
    os.environ.setdefault("JAX_PLATFORMS", "axon")
    from concourse.bass_utils import run_bass_kernel_spmd

    n_steps = NSTEP
    key = ("prog", n_steps)
    if key not in _CACHE:
        _CACHE[key] = _build_program(n_steps)
    nc = _CACHE[key]

    in_maps = _host_prep(inputs, n_steps)
    res = run_bass_kernel_spmd(nc, in_maps, list(range(N_CORES)))
    return _assemble(res.results, n_steps).reshape(-1, T)
